# revision 1
# baseline (speedup 1.0000x reference)
"""Bass/Trainium2 kernel for nn_DeepMPDRModel (8-core SPMD, batch-sharded).

Math (per half p in {r,i}, s=[B,181], v=[B,64]):
    w  = v @ phi3.T                  -> feature-major: wT = phi3 @ vT
    sw = s * w
    t  = sw @ (phi1@phi2).T = sw@M.T -> tT = M @ swT
    b  = s @ C,  C = phi4.T * phi5   -> bT = C.T(row-view) @ sT
    out = t * b ; global min/max norm (per half) ; swish-gate ; |complex|

Layout: feature-major on chip (features on partitions, batch on free axis).
Feature dim 181 is split into chunk0 = feats[0:128] (partitions 0:128) and
tail = feats[117:181] (64 wide, partitions 64:128; feats 117:128 are computed
twice, with the overlap rows of the K-tile-B weights zeroed so contractions
stay exact).  Inputs are cast to bf16 on the host and loaded transposed via
the DMA XBAR (16-bit lanes).
"""

import os
import sys

import numpy as np

try:  # make concourse importable when run standalone
    import concourse  # noqa: F401
except ImportError:
    for p in ("/opt/trn_rl_repo", "/root/.axon_site/_ro/trn_rl_repo"):
        if os.path.isdir(p):
            sys.path.insert(0, p)
            break

N_GRID = 181
N_ANT = 64
B_FULL = 65536
N_CORES = 8
LAST_EXEC_NS = None
LAST_TRACE = None

CH0 = 128            # chunk0 feature count
TL0 = 117            # tail feature start
TLW = 64             # tail width (feats 117:181)
TOV = 11             # overlap rows (117:128) zeroed in K-tile-B weights
NEG_INF = -3.0e38
POS_INF = 3.0e38


def _bf16(x):
    import ml_dtypes
    return np.asarray(x, dtype=np.float32).astype(ml_dtypes.bfloat16)


def build_host_params(phi1, phi2, phi3, phi4, phi5):
    """Pre-pack the tiny (<=181x181) parameter matrices for the kernel."""
    M = (phi1.astype(np.float64) @ phi2.astype(np.float64)).astype(np.float32)
    C = (phi4.T * phi5).astype(np.float32)          # [181,181]; b = s @ C
    MT = M.T.copy()                                  # lhsT for tT = M @ swT
    p3T = phi3.T.copy()                              # [64,181] lhsT for wT

    G = N_GRID
    # w-matmul stationary: rows 0:64 for r, rows 64:128 for i (row packing)
    p3T_A = np.zeros((128, CH0), np.float32)
    p3T_A[0:64] = p3T[:, 0:CH0]
    p3T_A[64:128] = p3T[:, 0:CH0]
    p3T_B = np.zeros((128, TLW), np.float32)
    p3T_B[0:64] = p3T[:, TL0:G]
    p3T_B[64:128] = p3T[:, TL0:G]

    def ktiles(L):  # L: [181,181] lhsT (k, m)
        A0 = L[0:CH0, 0:CH0]                        # K-tile A, M-chunk 0
        A1 = L[0:CH0, TL0:G]                        # K-tile A, M-chunk tail
        B0 = np.zeros((128, CH0), np.float32)       # K-tile B at partitions 64:128
        B1 = np.zeros((128, TLW), np.float32)
        kb = L[TL0:G, :].copy()                     # rows 117:181
        kb[0:TOV, :] = 0.0                          # zero the duplicated rows
        B0[64:128, :] = kb[:, 0:CH0]
        B1[64:128, :] = kb[:, TL0:G]
        return A0, A1, B0, B1

    MT_A0, MT_A1, MT_B0, MT_B1 = ktiles(MT)
    C_A0, C_A1, C_B0, C_B1 = ktiles(C)

    I128 = np.eye(128, dtype=np.float32)
    I64 = np.zeros((128, 64), np.float32)
    I64[64:128] = np.eye(64, dtype=np.float32)

    params = {
        "p3T_A": p3T_A, "p3T_B": p3T_B,
        "MT_A0": MT_A0, "MT_A1": MT_A1, "MT_B0": MT_B0, "MT_B1": MT_B1,
        "C_A0": C_A0, "C_A1": C_A1, "C_B0": C_B0, "C_B1": C_B1,
        "I128_bf": I128, "I64_bf": I64,
    }
    out = {k: np.ascontiguousarray(_bf16(v)) for k, v in params.items()}
    out["I128_f32"] = np.ascontiguousarray(I128)
    return out


def build_bass(b_loc, n_cores, nt_cols=512):
    """Build the per-core Bass program. Returns nc."""
    import os as _os
    _skip = set(_os.environ.get("KOPT_SKIP", "").split(","))
    from contextlib import ExitStack

    import concourse.bass as bass
    import concourse.tile as tile
    from concourse import mybir
    from concourse.bacc import Bacc

    NT = b_loc // nt_cols
    assert NT * nt_cols == b_loc and NT % 4 == 0
    G = N_GRID
    f32 = mybir.dt.float32
    bf16 = mybir.dt.bfloat16
    u16 = mybir.dt.uint16
    mult = mybir.AluOpType.mult
    addop = mybir.AluOpType.add
    subop = mybir.AluOpType.subtract
    maxop = mybir.AluOpType.max
    minop = mybir.AluOpType.min
    AF = mybir.ActivationFunctionType

    nc = Bacc("TRN2", target_bir_lowering=False, debug=False,
              num_devices=n_cores)

    # ---- DRAM I/O ----
    s_bf = nc.dram_tensor("s_bf", [b_loc, 2 * G], bf16, kind="ExternalInput")
    v_bf = nc.dram_tensor("v_bf", [b_loc, 2 * N_ANT], bf16, kind="ExternalInput")
    alpha_b = nc.dram_tensor("alpha_b", [128, 1], f32, kind="ExternalInput")
    beta_b = nc.dram_tensor("beta_b", [128, 1], f32, kind="ExternalInput")
    pnames = ["p3T_A", "p3T_B", "MT_A0", "MT_A1", "MT_B0", "MT_B1",
              "C_A0", "C_A1", "C_B0", "C_B1", "I128_bf", "I64_bf"]
    pshapes = {"p3T_A": [128, CH0], "p3T_B": [128, TLW],
               "MT_A0": [CH0, CH0], "MT_A1": [CH0, TLW],
               "MT_B0": [128, CH0], "MT_B1": [128, TLW],
               "C_A0": [CH0, CH0], "C_A1": [CH0, TLW],
               "C_B0": [128, CH0], "C_B1": [128, TLW],
               "I128_bf": [128, 128], "I64_bf": [128, 64]}
    pdram = {n: nc.dram_tensor(n, pshapes[n], bf16, kind="ExternalInput")
             for n in pnames}
    i128f = nc.dram_tensor("I128_f32", [128, 128], f32, kind="ExternalInput")
    out_d = nc.dram_tensor("out", [b_loc, G], f32, kind="ExternalOutput")

    cc_in = nc.dram_tensor("cc_in", [4], f32, kind="Internal")
    cc_out = nc.dram_tensor("cc_out", [4], f32, kind="Internal",
                            addr_space="Shared")

    with tile.TileContext(nc) as tc, ExitStack() as ctx:
        const = ctx.enter_context(tc.tile_pool(name="const", bufs=1))

        # ---- load params ----
        sb_p = {}
        for n in pnames:
            t = const.tile(pshapes[n], bf16, name=n, tag=n)
            nc.sync.dma_start(out=t[:], in_=pdram[n].ap())
            sb_p[n] = t
        sb_i128f = const.tile([128, 128], f32, name="i128f", tag="i128f")
        nc.sync.dma_start(out=sb_i128f[:], in_=i128f.ap())
        sb_alpha = const.tile([128, 1], f32, name="alpha", tag="alpha")
        nc.sync.dma_start(out=sb_alpha[:], in_=alpha_b.ap())
        sb_beta = const.tile([128, 1], f32, name="beta", tag="beta")
        nc.sync.dma_start(out=sb_beta[:], in_=beta_b.ap())

        # ---- persistent phase-1/2 tensors ----
        big = ctx.enter_context(tc.tile_pool(name="big", bufs=1))
        prodA = {p: big.tile([128, b_loc], bf16, name=f"prodA_{p}", tag=f"prodA_{p}")
                 for p in ("r", "i")}
        # tails: rows 64:128; r at cols 0:b_loc, i at cols b_loc:2*b_loc
        prodT = big.tile([128, 2 * b_loc], bf16, name="prodT", tag="prodT")
        accmax = {p: const.tile([128, 2 * (NT // 4 + 1)], f32,
                                name=f"accmax_{p}", tag=f"accmax_{p}")
                  for p in ("r", "i")}
        # min: DVE reduce over 4-N-tile spans (free-dim reduce is DVE-only;
        # this toolchain has no POOL tensor_tensor)
        accmin = {p: const.tile([128, 2 * (NT // 4 + 1)], f32,
                                name=f"accmin_{p}", tag=f"accmin_{p}")
                  for p in ("r", "i")}
        for p in ("r", "i"):
            nc.vector.memset(accmax[p][:], NEG_INF)
            nc.vector.memset(accmin[p][:], POS_INF)

        svec = {}  # derived per-partition scalar vectors, filled later
        for p in ("r", "i"):
            for nm in ("mn", "inv", "sc0", "bi0"):
                svec[(p, nm)] = const.tile([128, 1], f32, name=f"sv_{p}_{nm}", tag=f"sv_{p}_{nm}")

        # ================= PHASE 1 =================
        with ExitStack() as p1:
            sT = p1.enter_context(tc.tile_pool(name="sT", bufs=1))
            work = p1.enter_context(tc.tile_pool(name="work", bufs=3))
            psum = p1.enter_context(tc.tile_pool(name="psum", bufs=8,
                                                 space="PSUM"))

            # ---- transposed bf16 loads via the DMA XBAR ----
            sv = s_bf.ap()
            vv = v_bf.ap()

            def tload(name, src_view, f0, f1):
                t = sT.tile([128, b_loc], bf16, name=name, tag=name)
                nc.sync.dma_start_transpose(t[:], src_view[:, f0:f1])
                return t

            vT = tload("vT", vv, 0, 128)             # parts 0:64 r, 64:128 i
            sA = {}
            sB = {}
            sA["r"] = tload("sA_r", sv, 0, CH0)
            sB["r"] = tload("sB_r", sv, TL0 - 64, TL0 + 64)
            sA["i"] = tload("sA_i", sv, G, G + CH0)
            sB["i"] = tload("sB_i", sv, G + TL0 - 64, G + TL0 + 64)
            # sB: partitions 64:128 = feats 117:181 of that half

            for n in range(NT):
                cols = slice(n * nt_cols, (n + 1) * nt_cols)
                wA, wB, swA, swB, bA, bB, bsA, bsB, tA, tB = \
                    {}, {}, {}, {}, {}, {}, {}, {}, {}, {}
                for ip, p in enumerate(("r", "i")):
                    vb = 64 * ip
                    wA[p] = psum.tile([128, nt_cols], f32, name="wA", tag="ps")
                    nc.tensor.matmul(wA[p][:], sb_p["p3T_A"][vb:vb + 64, :],
                                     vT[vb:vb + 64, cols])
                    wB[p] = psum.tile([128, nt_cols], f32, name="wB", tag="ps")
                    nc.tensor.matmul(wB[p][64:128, :],
                                     sb_p["p3T_B"][vb:vb + 64, :],
                                     vT[vb:vb + 64, cols])
                for p in ("r", "i"):
                    swA[p] = work.tile([128, nt_cols], bf16, name="swA",
                                       tag="swA")
                    swB[p] = work.tile([128, nt_cols], bf16, name="swB",
                                       tag="swB")
                    if "swprod" not in _skip:
                        nc.vector.tensor_tensor(swA[p][:], sA[p][:, cols],
                                                wA[p][:], mult)
                        nc.vector.tensor_tensor(swB[p][64:128, :],
                                                sB[p][64:128, cols],
                                                wB[p][64:128, :], mult)
                    else:
                        nc.vector.memset(swA[p][:], 0.5)
                        nc.vector.memset(swB[p][64:128, :], 0.5)
                for p in ("r", "i"):
                    bA[p] = psum.tile([128, nt_cols], f32, name="bA", tag="ps")
                    nc.tensor.matmul(bA[p][:], sb_p["C_A0"][:], sA[p][:, cols],
                                     start=True, stop=False)
                    nc.tensor.matmul(bA[p][:], sb_p["C_B0"][64:128, :],
                                     sB[p][64:128, cols],
                                     start=False, stop=True)
                    bB[p] = psum.tile([128, nt_cols], f32, name="bB", tag="ps")
                    nc.tensor.matmul(bB[p][64:128, :], sb_p["C_A1"][:],
                                     sA[p][:, cols], start=True, stop=False)
                    nc.tensor.matmul(bB[p][64:128, :], sb_p["C_B1"][64:128, :],
                                     sB[p][64:128, cols],
                                     start=False, stop=True)
                for p in ("r", "i"):
                    bsA[p] = work.tile([128, nt_cols], bf16, name="bsA",
                                       tag="bsA")
                    bsB[p] = work.tile([128, nt_cols], bf16, name="bsB",
                                       tag="bsB")
                    if "swprod" not in _skip:
                        nc.scalar.copy(bsA[p][:], bA[p][:])
                        nc.scalar.copy(bsB[p][64:128, :], bB[p][64:128, :])
                for p in ("r", "i"):
                    tA[p] = psum.tile([128, nt_cols], f32, name="tA", tag="ps")
                    nc.tensor.matmul(tA[p][:], sb_p["MT_A0"][:], swA[p][:],
                                     start=True, stop=False)
                    nc.tensor.matmul(tA[p][:], sb_p["MT_B0"][64:128, :],
                                     swB[p][64:128, :], start=False, stop=True)
                    tB[p] = psum.tile([128, nt_cols], f32, name="tB", tag="ps")
                    nc.tensor.matmul(tB[p][64:128, :], sb_p["MT_A1"][:],
                                     swA[p][:], start=True, stop=False)
                    nc.tensor.matmul(tB[p][64:128, :], sb_p["MT_B1"][64:128, :],
                                     swB[p][64:128, :], start=False, stop=True)
                for ip, p in enumerate(("r", "i")):
                    tcols = slice(ip * b_loc + n * nt_cols,
                                  ip * b_loc + (n + 1) * nt_cols)
                    if "swprod" not in _skip:
                        nc.vector.tensor_tensor(prodA[p][:, cols], tA[p][:],
                                                bsA[p][:], mult)
                        nc.vector.tensor_tensor(prodT[64:128, tcols],
                                                tB[p][64:128, :],
                                                bsB[p][64:128, :], mult)
                    if n % 4 == 3 and "reduces" not in _skip:
                        g4 = n // 4
                        W4 = 4 * nt_cols
                        c0 = (n - 3) * nt_cols
                        t0 = ip * b_loc + (n - 3) * nt_cols
                        for (srcT, off, rows, accc) in (
                                (prodA[p], c0, slice(0, 128), g4),
                                (prodT, t0, slice(64, 128), NT // 4 + g4)):
                            w = W4 // 2
                            scr = work.tile([128, W4 // 2], bf16,
                                            name="tscr", tag="tscr")
                            nc.vector.tensor_tensor(
                                scr[rows, 0:w], srcT[rows, off:off + w],
                                srcT[rows, off + w:off + 2 * w], maxop)
                            scr2 = work.tile([128, W4 // 2], bf16,
                                             name="tscr2", tag="tscr2")
                            nc.vector.tensor_tensor(
                                scr2[rows, 0:w], srcT[rows, off:off + w],
                                srcT[rows, off + w:off + 2 * w], minop)
                            while w > 128:
                                w //= 2
                                nc.vector.tensor_tensor(
                                    scr[rows, 0:w], scr[rows, 0:w],
                                    scr[rows, w:2 * w], maxop)
                                nc.vector.tensor_tensor(
                                    scr2[rows, 0:w], scr2[rows, 0:w],
                                    scr2[rows, w:2 * w], minop)
                            nc.vector.tensor_reduce(
                                accmax[p][rows, accc:accc + 1],
                                scr[rows, 0:128],
                                axis=mybir.AxisListType.X, op=maxop)
                            nc.vector.tensor_reduce(
                                accmin[p][rows, accc:accc + 1],
                                scr2[rows, 0:128],
                                axis=mybir.AxisListType.X, op=minop)

            # ---- local min/max -> 4 scalars -> AllReduce(max) ----
            acc4 = work.tile([128, 4], f32, name="acc4", tag="acc4")
            nc.vector.tensor_reduce(acc4[:, 0:1], accmax["r"][:],
                                    axis=mybir.AxisListType.X, op=maxop)
            nc.vector.tensor_reduce(acc4[:, 1:2], accmax["i"][:],
                                    axis=mybir.AxisListType.X, op=maxop)
            for ip, p in enumerate(("r", "i")):
                negm = work.tile([128, 2 * (NT // 4 + 1)], f32,
                                 name="negm", tag="negm")
                nc.vector.tensor_scalar(negm[:], accmin[p][:], -1.0, None, mult)
                nc.vector.tensor_reduce(acc4[:, 2 + ip:3 + ip], negm[:],
                                        axis=mybir.AxisListType.X, op=maxop)
            acc4T = psum.tile([128, 512], f32, name="ps", tag="ps")
            nc.tensor.transpose(acc4T[0:4, 0:128], acc4[:], sb_i128f[:])
            loc4 = work.tile([4, 1], f32, name="loc4", tag="loc4")
            nc.vector.tensor_reduce(loc4[:], acc4T[0:4, 0:128],
                                    axis=mybir.AxisListType.X, op=maxop)
            nc.sync.dma_start(out=cc_in.ap(), in_=loc4[:])
            if n_cores > 1:
                nc.gpsimd.collective_compute(
                    "AllReduce", maxop,
                    replica_groups=[list(range(n_cores))],
                    ins=[cc_in.ap()], outs=[cc_out.ap()])
                cc_res = cc_out
            else:
                cc_res = cc_in
            # broadcast the 4 reduced scalars to all partitions via a
            # partition-stride-0 DMA read
            globb = work.tile([128, 4], f32, name="globb", tag="globb")
            bcast = bass.AP(tensor=cc_res.ap().tensor, offset=0,
                            ap=[[0, 128], [1, 4]])
            nc.sync.dma_start(out=globb[:], in_=bcast)

            # derived per-partition scalars for each half
            tmp = work.tile([128, 1], f32, name="tmp", tag="tmp")
            tmp2 = work.tile([128, 1], f32, name="tmp2", tag="tmp2")
            for ip, p in enumerate(("r", "i")):
                mx = globb[:, ip:ip + 1]
                ngmn = globb[:, 2 + ip:3 + ip]
                nc.vector.tensor_scalar(svec[(p, "mn")][:], ngmn, -1.0, None, mult)
                nc.vector.tensor_tensor(tmp[:], mx, ngmn, addop)  # mx - mn
                nc.vector.reciprocal(svec[(p, "inv")][:], tmp[:])
                nc.vector.tensor_tensor(svec[(p, "sc0")][:],
                                        svec[(p, "inv")][:], sb_alpha[:], mult)
                nc.vector.tensor_tensor(tmp[:], svec[(p, "mn")][:],
                                        svec[(p, "inv")][:], mult)
                nc.vector.tensor_tensor(tmp2[:], tmp[:], sb_beta[:], addop)
                nc.vector.scalar_tensor_tensor(
                    svec[(p, "bi0")][:], in0=tmp2[:], scalar=-1.0,
                    in1=sb_alpha[:], op0=mult, op1=mult)

        # ================= PHASE 2 =================
        with ExitStack() as p2:
            gp = p2.enter_context(tc.tile_pool(name="gp", bufs=2))
            ops = p2.enter_context(tc.tile_pool(name="ops", bufs=2))
            psum2 = p2.enter_context(tc.tile_pool(name="psum2", bufs=2,
                                                  space="PSUM"))
            HW = b_loc // 4
            for h in range(0 if "phase2" in _skip else 4):
                hc = slice(h * HW, (h + 1) * HW)
                for ip, p in enumerate(("r", "i")):
                    tc_ = slice(ip * b_loc + h * HW, ip * b_loc + (h + 1) * HW)
                    g0 = gp.tile([128, HW], bf16, name="g0", tag="g0")
                    nc.scalar.activation(g0[:], prodA[p][:, hc], AF.Sigmoid,
                                         bias=svec[(p, "bi0")][:],
                                         scale=svec[(p, "sc0")][:])
                    gt = gp.tile([128, HW], bf16, name="gt", tag="gt")
                    nc.scalar.activation(gt[64:128, :], prodT[64:128, tc_],
                                         AF.Sigmoid,
                                         bias=svec[(p, "bi0")][64:128, :],
                                         scale=svec[(p, "sc0")][64:128, :])
                    # s_norm into scratch (runs concurrent with the ACT
                    # sigmoid instead of WAR-serializing behind it), then
                    # res = s_norm * g and square in place
                    snb = gp.tile([128, HW], bf16, name="snb", tag="snb")
                    nc.vector.tensor_scalar(snb[:], prodA[p][:, hc],
                                            svec[(p, "mn")][:],
                                            svec[(p, "inv")][:], subop, mult)
                    nc.vector.tensor_tensor(prodA[p][:, hc], snb[:],
                                            g0[:], mult)
                    nc.vector.tensor_tensor(prodA[p][:, hc], prodA[p][:, hc],
                                            prodA[p][:, hc], mult)
                    snt = gp.tile([128, HW], bf16, name="snt", tag="snt")
                    nc.vector.tensor_scalar(snt[64:128, :],
                                            prodT[64:128, tc_],
                                            svec[(p, "mn")][64:128, :],
                                            svec[(p, "inv")][64:128, :], subop, mult)
                    nc.vector.tensor_tensor(prodT[64:128, tc_],
                                            snt[64:128, :],
                                            gt[64:128, :], mult)
                    nc.vector.tensor_tensor(prodT[64:128, tc_],
                                            prodT[64:128, tc_],
                                            prodT[64:128, tc_], mult)
                # sumsq = sq_r + sq_i (into the r buffers)
                nc.vector.tensor_tensor(prodA["r"][:, hc], prodA["r"][:, hc],
                                        prodA["i"][:, hc], addop)
                nc.vector.tensor_tensor(
                    prodT[64:128, slice(h * HW, (h + 1) * HW)],
                    prodT[64:128, slice(h * HW, (h + 1) * HW)],
                    prodT[64:128, slice(b_loc + h * HW, b_loc + (h + 1) * HW)],
                    addop)

            # ---- transpose back to batch-major, fused sqrt, store ----
            NB = b_loc // 128          # 128-row batch blocks
            out_v = out_d.ap().rearrange("(g j p) f -> g p j f", p=128, j=8)
            for gidx in range(0 if "outxf" in _skip else NB // 8):
                po = psum2.tile([128, 8, 512], bf16, name="po", tag="po")
                for j in range(8):
                    blk = gidx * 8 + j
                    bc = slice(blk * 128, (blk + 1) * 128)
                    nc.tensor.transpose(po[:, j, 0:128],
                                        prodA["r"][:, bc], sb_p["I128_bf"][:])
                    nc.tensor.transpose(po[:, j, 128:192],
                                        prodT[64:128, bc],
                                        sb_p["I64_bf"][64:128, :])
                ot = ops.tile([128, 8, G], f32, name="ot", tag="ot")
                nc.scalar.activation(ot[:, :, 0:CH0], po[:, :, 0:CH0], AF.Sqrt)
                nc.scalar.activation(ot[:, :, CH0:G],
                                     po[:, :, CH0 + TOV:192], AF.Sqrt)
                nc.sync.dma_start(out=out_v[gidx], in_=ot[:])

    nc.compile()
    return nc


def kernel(**inputs):
    s_minus = np.ascontiguousarray(inputs["s_minus"], dtype=np.float32)
    bf_vector = np.ascontiguousarray(inputs["bf_vector"], dtype=np.float32)
    alpha = np.asarray(inputs["alpha"], dtype=np.float32).reshape(-1)[0]
    beta = np.asarray(inputs["beta"], dtype=np.float32).reshape(-1)[0]

    params = build_host_params(
        np.asarray(inputs["phi1"], np.float32), np.asarray(inputs["phi2"], np.float32),
        np.asarray(inputs["phi3"], np.float32), np.asarray(inputs["phi4"], np.float32),
        np.asarray(inputs["phi5"], np.float32))

    import ml_dtypes
    s_bf_full = np.ascontiguousarray(s_minus.astype(ml_dtypes.bfloat16))
    v_bf_full = np.ascontiguousarray(bf_vector.astype(ml_dtypes.bfloat16))
    b_loc = B_FULL // N_CORES
    nc = build_bass(b_loc, N_CORES)

    base = {k: np.ascontiguousarray(np.asarray(v).view(np.uint16)
                                    if v.dtype == ml_dtypes.bfloat16 else v)
            for k, v in params.items()}
    # bf16 tensors must be passed with bf16 dtype
    base = {}
    for k, v in params.items():
        base[k] = np.ascontiguousarray(v)
    base["alpha_b"] = np.full((128, 1), alpha, np.float32)
    base["beta_b"] = np.full((128, 1), beta, np.float32)

    in_maps = []
    for c in range(N_CORES):
        m = dict(base)
        m["s_bf"] = s_bf_full[c * b_loc:(c + 1) * b_loc]
        m["v_bf"] = v_bf_full[c * b_loc:(c + 1) * b_loc]
        in_maps.append(m)

    from concourse.bass_utils import run_bass_kernel_spmd
    res = run_bass_kernel_spmd(nc, in_maps, core_ids=list(range(N_CORES)))
    global LAST_EXEC_NS, LAST_TRACE
    LAST_EXEC_NS = res.exec_time_ns
    LAST_TRACE = res.instructions_and_trace[1] if res.instructions_and_trace else None
    if LAST_EXEC_NS is not None:
        print(f"HW exec time: {LAST_EXEC_NS} ns")
    if LAST_TRACE:
        print(f"trace: {LAST_TRACE}")
    return np.concatenate([r["out"] for r in res.results], axis=0)



# revision 22
# speedup vs baseline: 1.5176x; 1.5176x over previous
"""Bass/Trainium2 kernel for nn_DeepMPDRModel (8-core SPMD, batch-sharded).

Math (per half p in {r,i}, s=[B,181], v=[B,64]):
    w  = v @ phi3.T                  -> feature-major: wT = phi3 @ vT
    sw = s * w
    t  = sw @ (phi1@phi2).T = sw@M.T -> tT = M @ swT
    b  = s @ C,  C = phi4.T * phi5   -> bT = C.T(row-view) @ sT
    out = t * b ; global min/max norm (per half) ; swish-gate ; |complex|

Layout: feature-major on chip (features on partitions, batch on free axis).
Features split into a main chunk (feats 0:128, one 128-partition stream per
half) and ONE merged tail stream (feats 117:181 of BOTH halves in a single
128-partition tile: partitions 0:64 = i-half tail, 64:128 = r-half tail;
the tail input s_tail comes host-packed in that order).  K-tile-B weights
have their first TOV=11 rows zeroed so the A/B-chunk contractions stay
exact.

Two-pass schedule to hide the global min/max collectives (fixed ~15us
AllGather latency each):
  pass A: everything for the r half + the whole merged tail stream
          (incl. the i-half w/sw, which the tail t-matmuls consume), the
          r/tail min-max trees -> AllGather(r extrema) runs during pass B
  pass B: b/t/prod for the i-main stream + its trees
          -> AllGather(i extrema) runs during phase2-r
  phase2-r: normalization + gate + square for the r-main stream
  phase2-rest: same for i-main + merged tail, |.|, sqrt, stores

Engine plan per 512-col batch tile: PE computes w/b/t into PSUM; ACT
copies w and b PSUM->bf16 (hardware allows at most one PSUM operand per
DVE op); DVE does sw = s*w in 2x bf16 mode and prod = t*b at 1x; min/max
run as bf16 2x pairwise trees on DVE, emission-interleaved across tiles.
Phase 2 runs ts at 4x / TT at 2x bf16, sigmoids pre-issued per phase so
the in-order ACT queue cannot head-of-line block DVE, sqrt on ACT, bf16
feature-major stores (outA [128,B], outT [64,B]) reassembled to [B,181]
f32 on the host.  All DMAs go through the SP queue (ACT-queue DMAs
proved racy on hardware).
"""

import os
import sys

import numpy as np

try:  # make concourse importable when run standalone
    import concourse  # noqa: F401
except ImportError:
    for p in ("/opt/trn_rl_repo", "/root/.axon_site/_ro/trn_rl_repo"):
        if os.path.isdir(p):
            sys.path.insert(0, p)
            break

N_GRID = 181
N_ANT = 64
B_FULL = 65536
N_CORES = 8
LAST_EXEC_NS = None

CH0 = 128            # main-chunk feature count
TL0 = 117            # tail feature start
TLW = 64             # tail width (feats 117:181)
TOV = 11             # K-overlap rows (117:128) zeroed in K-tile-B weights
NEG_INF = -3.0e38


def _bf16(x):
    import ml_dtypes
    return np.asarray(x, dtype=np.float32).astype(ml_dtypes.bfloat16)


def build_host_params(phi1, phi2, phi3, phi4, phi5):
    """Pre-pack the tiny (<=181x181) parameter matrices for the kernel."""
    M = (phi1.astype(np.float64) @ phi2.astype(np.float64)).astype(np.float32)
    C = (phi4.T * phi5).astype(np.float32)          # [181,181]; b = s @ C
    MT = M.T.copy()                                  # lhsT for tT = M @ swT
    p3T = phi3.T.copy()                              # [64,181] lhsT for wT

    G = N_GRID
    # w-matmul stationary: same [64,...] block stacked at rows 0:64 (for
    # moving data on partitions 0:64) and 64:128 (for partitions 64:128).
    p3T_A = np.zeros((128, CH0), np.float32)
    p3T_A[0:64] = p3T[:, 0:CH0]
    p3T_A[64:128] = p3T[:, 0:CH0]
    p3T_B = np.zeros((128, TLW), np.float32)
    p3T_B[0:64] = p3T[:, TL0:G]
    p3T_B[64:128] = p3T[:, TL0:G]

    def ktiles(L):  # L: [181,181] lhsT (k, m)
        A0 = L[0:CH0, 0:CH0]                        # K-tile A, M main
        A1 = L[0:CH0, TL0:G]                        # K-tile A, M tail
        kb = L[TL0:G, :].copy()                     # K rows 117:181
        kb[0:TOV, :] = 0.0                          # zero K-overlap rows
        B0 = np.zeros((128, CH0), np.float32)       # K-tile B, stacked 2x
        B0[0:64] = kb[:, 0:CH0]
        B0[64:128] = kb[:, 0:CH0]
        B1 = np.zeros((128, TLW), np.float32)
        B1[0:64] = kb[:, TL0:G]
        B1[64:128] = kb[:, TL0:G]
        return A0, A1, B0, B1

    MT_A0, MT_A1, MT_B0, MT_B1 = ktiles(MT)
    C_A0, C_A1, C_B0, C_B1 = ktiles(C)

    params = {
        "p3T_A": p3T_A, "p3T_B": p3T_B,
        "MT_A0": MT_A0, "MT_A1": MT_A1, "MT_B0": MT_B0, "MT_B1": MT_B1,
        "C_A0": C_A0, "C_A1": C_A1, "C_B0": C_B0, "C_B1": C_B1,
    }
    out = {k: np.ascontiguousarray(_bf16(v)) for k, v in params.items()}
    out["I128_f32"] = np.ascontiguousarray(np.eye(128, dtype=np.float32))
    return out


PSHAPES = {"p3T_A": [128, CH0], "p3T_B": [128, TLW],
           "MT_A0": [CH0, CH0], "MT_A1": [CH0, TLW],
           "MT_B0": [128, CH0], "MT_B1": [128, TLW],
           "C_A0": [CH0, CH0], "C_A1": [CH0, TLW],
           "C_B0": [128, CH0], "C_B1": [128, TLW]}


def build_bass(b_loc, n_cores, nt_cols=512):
    """Build the per-core Bass program. Returns nc."""
    from contextlib import ExitStack

    import concourse.bass as bass
    import concourse.tile as tile
    from concourse import mybir
    from concourse.bacc import Bacc

    NT = b_loc // nt_cols
    assert NT * nt_cols == b_loc and NT % 4 == 0
    NCH = min(4, NT)              # input load chunks
    TPC = NT // NCH               # batch tiles per chunk
    CW = b_loc // NCH             # chunk col width
    GRP = 8 if NT % 8 == 0 else 4  # min/max tree group size (tiles)
    NG = NT // GRP
    GW = GRP * nt_cols
    G = N_GRID
    f32 = mybir.dt.float32
    bf16 = mybir.dt.bfloat16
    mult = mybir.AluOpType.mult
    addop = mybir.AluOpType.add
    subop = mybir.AluOpType.subtract
    maxop = mybir.AluOpType.max
    minop = mybir.AluOpType.min
    AF = mybir.ActivationFunctionType

    nc = Bacc("TRN2", target_bir_lowering=False, debug=False,
              num_devices=n_cores)

    # ---- DRAM I/O ----
    s_bf = nc.dram_tensor("s_bf", [b_loc, 2 * G], bf16, kind="ExternalInput")
    v_bf = nc.dram_tensor("v_bf", [b_loc, 2 * N_ANT], bf16,
                          kind="ExternalInput")
    st_bf = nc.dram_tensor("st_bf", [b_loc, 128], bf16,
                           kind="ExternalInput")
    alpha_b = nc.dram_tensor("alpha_b", [128, 1], f32, kind="ExternalInput")
    beta_b = nc.dram_tensor("beta_b", [128, 1], f32, kind="ExternalInput")
    pdram = {n: nc.dram_tensor(n, PSHAPES[n], bf16, kind="ExternalInput")
             for n in PSHAPES}
    i128f = nc.dram_tensor("I128_f32", [128, 128], f32, kind="ExternalInput")
    outA_d = nc.dram_tensor("outA", [128, b_loc], bf16,
                            kind="ExternalOutput")
    outT_d = nc.dram_tensor("outT", [TLW, b_loc], bf16,
                            kind="ExternalOutput")

    cc_in = {p: nc.dram_tensor(f"cc_in_{p}", [4], f32, kind="Internal")
             for p in ("r", "i")}
    cc_out = {p: nc.dram_tensor(f"cc_out_{p}", [4 * n_cores], f32,
                                kind="Internal", addr_space="Shared")
              for p in ("r", "i")}

    HF = {"r": slice(64, 128), "i": slice(0, 64)}  # tail partition ranges

    with tile.TileContext(nc) as tc, ExitStack() as ctx:
        const = ctx.enter_context(tc.tile_pool(name="const", bufs=1))

        # ---- params (w weights first: first matmuls need them) ----
        sb_p = {}

        def load_param(n):
            t = const.tile(PSHAPES[n], bf16, name=n, tag=n)
            nc.sync.dma_start(out=t[:], in_=pdram[n].ap())
            sb_p[n] = t

        for n in ("p3T_A", "p3T_B"):
            load_param(n)

        # ---- persistent outputs of phase 1 ----
        big = ctx.enter_context(tc.tile_pool(name="big", bufs=1))
        prodA = {p: big.tile([128, b_loc], bf16, name=f"prodA_{p}",
                             tag=f"prodA_{p}") for p in ("r", "i")}
        prodT = big.tile([128, b_loc], bf16, name="prodT", tag="prodT")
        accmax = {s: const.tile([128, NG], f32, name=f"accmax_{s}",
                                tag=f"accmax_{s}") for s in ("r", "i", "t")}
        accmin = {s: const.tile([128, NG], f32, name=f"accmin_{s}",
                                tag=f"accmin_{s}") for s in ("r", "i", "t")}

        svec = {}
        for p in ("r", "i", "t"):
            for nm in ("mn", "inv", "sc0", "bi0"):
                svec[(p, nm)] = const.tile([128, 1], f32,
                                           name=f"sv_{p}_{nm}",
                                           tag=f"sv_{p}_{nm}")
        sb_i128f = const.tile([128, 128], f32, name="i128f", tag="i128f")
        sb_alpha = const.tile([128, 1], f32, name="alpha", tag="alpha")
        sb_beta = const.tile([128, 1], f32, name="beta", tag="beta")

        with ExitStack() as p1:
            work = p1.enter_context(tc.tile_pool(name="work", bufs=3))
            p1i = p1.enter_context(ExitStack())
            # ---- chunked transposed loads (bf16 via DMA XBAR, SP only) ----
            inp = p1i.enter_context(tc.tile_pool(name="inp", bufs=1))
            sv = s_bf.ap()
            vv = v_bf.ap()
            vT, sAr, sAi, sB = [], [], [], []
            for c in range(NCH):
                rows = slice(c * CW, (c + 1) * CW)
                t_v = inp.tile([128, CW], bf16, name=f"vT{c}", tag=f"vT{c}")
                nc.sync.dma_start_transpose(t_v[:], vv[rows, 0:128])
                vT.append(t_v)
                t_sr = inp.tile([128, CW], bf16, name=f"sAr{c}",
                                tag=f"sAr{c}")
                nc.sync.dma_start_transpose(t_sr[:], sv[rows, 0:CH0])
                sAr.append(t_sr)
                # tails host-packed: st_bf cols 0:64 = i tail, 64:128 = r
                t_sb = inp.tile([128, CW], bf16, name=f"sB{c}", tag=f"sB{c}")
                nc.sync.dma_start_transpose(t_sb[:],
                                            st_bf.ap()[rows, 0:128])
                sB.append(t_sb)
                t_si = inp.tile([128, CW], bf16, name=f"sAi{c}",
                                tag=f"sAi{c}")
                nc.sync.dma_start_transpose(t_si[:], sv[rows, G:G + CH0])
                sAi.append(t_si)
                if c == 0:
                    for n in ("C_A0", "C_B0", "C_A1", "C_B1",
                              "MT_A0", "MT_B0", "MT_A1", "MT_B1"):
                        load_param(n)
                    nc.sync.dma_start(out=sb_i128f[:], in_=i128f.ap())
                    nc.sync.dma_start(out=sb_alpha[:], in_=alpha_b.ap())
                    nc.sync.dma_start(out=sb_beta[:], in_=beta_b.ap())
            sA = {"r": sAr, "i": sAi}

            # i-half sw persists into pass B (tail t-matmuls also use swT)
            swAi = p1i.enter_context(tc.tile_pool(name="swAi", bufs=1))
            swAi_big = swAi.tile([128, b_loc], bf16, name="swAi",
                                 tag="swAi")
            swT_big = swAi.tile([128, b_loc], bf16, name="swTb", tag="swTb")

            tree = p1i.enter_context(tc.tile_pool(name="tree", bufs=2))
            psum = p1i.enter_context(tc.tile_pool(name="psum", bufs=8,
                                                  space="PSUM"))

            # --- deferred min/max tree ops, interleaved across tiles ---
            pending = []

            def tree_thunks(s, src, g8, c0):
                def lvl(op, acc):
                    scr1 = {}

                    def first():
                        scr1[0] = tree.tile([128, GW // 2], bf16,
                                            name="tsc", tag=f"tsc_{s}")
                        w = GW // 2
                        nc.vector.tensor_tensor(
                            scr1[0][:, 0:w], src[:, c0:c0 + w],
                            src[:, c0 + w:c0 + 2 * w], op)
                    yield first
                    w = GW // 2
                    while w > 256:
                        w //= 2

                        def halve(w=w):
                            nc.vector.tensor_tensor(
                                scr1[0][:, 0:w], scr1[0][:, 0:w],
                                scr1[0][:, w:2 * w], op)
                        yield halve

                    def fin():
                        nc.vector.tensor_reduce(
                            acc[:, g8:g8 + 1], scr1[0][:, 0:256],
                            axis=mybir.AxisListType.X, op=op)
                    yield fin
                for op, acc in ((maxop, accmax[s]), (minop, accmin[s])):
                    for th in lvl(op, acc):
                        pending.append(th)

            def drain_pending(k):
                for _ in range(min(k, len(pending))):
                    pending.pop(0)()

            # ================= PASS A: r half + merged tail =================
            for n in range(NT):
                ch, lt = n // TPC, n % TPC
                lc = slice(lt * nt_cols, (lt + 1) * nt_cols)
                cols = slice(n * nt_cols, (n + 1) * nt_cols)
                # w matmuls -> ACT copies -> sw (2x bf16)
                wps = {}
                for p in ("r", "i"):
                    vb = 0 if p == "r" else 64
                    wps[p] = psum.tile([128, nt_cols], f32, name="wA",
                                       tag="ps")
                    nc.tensor.matmul(wps[p][:],
                                     sb_p["p3T_A"][vb:vb + 64, :],
                                     vT[ch][vb:vb + 64, lc])
                wps["t"] = psum.tile([128, nt_cols], f32, name="wT",
                                     tag="ps")
                nc.tensor.matmul(wps["t"][HF["r"], :],
                                 sb_p["p3T_B"][0:64, :], vT[ch][0:64, lc])
                nc.tensor.matmul(wps["t"][HF["i"], :],
                                 sb_p["p3T_B"][64:128, :],
                                 vT[ch][64:128, lc])
                wbf = {}
                for k in ("r", "i", "t"):
                    wbf[k] = work.tile([128, nt_cols], bf16, name="wbf",
                                       tag=f"wbf_{k}")
                    nc.scalar.copy(wbf[k][:], wps[k][:])
                swA_r = work.tile([128, nt_cols], bf16, name="swA",
                                  tag="swA_r")
                nc.vector.tensor_tensor(swA_r[:], sA["r"][ch][:, lc],
                                        wbf["r"][:], mult)
                nc.vector.tensor_tensor(swAi_big[:, cols],
                                        sA["i"][ch][:, lc], wbf["i"][:],
                                        mult)
                nc.vector.tensor_tensor(swT_big[:, cols], sB[ch][:, lc],
                                        wbf["t"][:], mult)
                # b matmuls (r + tail) -> ACT bf16 copies
                bA = psum.tile([128, nt_cols], f32, name="bA", tag="ps")
                nc.tensor.matmul(bA[:], sb_p["C_A0"][:], sA["r"][ch][:, lc],
                                 start=True, stop=False)
                nc.tensor.matmul(bA[:], sb_p["C_B0"][64:128, :],
                                 sB[ch][64:128, lc], start=False, stop=True)
                bbf_r = work.tile([128, nt_cols], bf16, name="bbf",
                                  tag="bbf_r")
                nc.scalar.copy(bbf_r[:], bA[:])
                bT = psum.tile([128, nt_cols], f32, name="bT", tag="ps")
                for p in ("r", "i"):
                    hf = HF[p]
                    nc.tensor.matmul(bT[hf, :], sb_p["C_A1"][:],
                                     sA[p][ch][:, lc], start=True,
                                     stop=False)
                    nc.tensor.matmul(bT[hf, :], sb_p["C_B1"][hf, :],
                                     sB[ch][hf, lc], start=False, stop=True)
                bbf_t = work.tile([128, nt_cols], bf16, name="bbf",
                                  tag="bbf_t")
                nc.scalar.copy(bbf_t[:], bT[:])
                # t matmuls (r + tail); prod = t*b
                tA = psum.tile([128, nt_cols], f32, name="tA", tag="ps")
                nc.tensor.matmul(tA[:], sb_p["MT_A0"][:], swA_r[:],
                                 start=True, stop=False)
                nc.tensor.matmul(tA[:], sb_p["MT_B0"][64:128, :],
                                 swT_big[64:128, cols], start=False,
                                 stop=True)
                nc.vector.tensor_tensor(prodA["r"][:, cols], tA[:],
                                        bbf_r[:], mult)
                tT = psum.tile([128, nt_cols], f32, name="tT", tag="ps")
                nc.tensor.matmul(tT[HF["r"], :], sb_p["MT_A1"][:], swA_r[:],
                                 start=True, stop=False)
                nc.tensor.matmul(tT[HF["r"], :], sb_p["MT_B1"][64:128, :],
                                 swT_big[64:128, cols], start=False,
                                 stop=True)
                nc.tensor.matmul(tT[HF["i"], :], sb_p["MT_A1"][:],
                                 swAi_big[:, cols], start=True, stop=False)
                nc.tensor.matmul(tT[HF["i"], :], sb_p["MT_B1"][0:64, :],
                                 swT_big[0:64, cols], start=False,
                                 stop=True)
                nc.vector.tensor_tensor(prodT[:, cols], tT[:], bbf_t[:],
                                        mult)
                drain_pending(3)
                if n % GRP == GRP - 1:
                    c0 = (n - GRP + 1) * nt_cols
                    tree_thunks("r", prodA["r"], n // GRP, c0)
                    tree_thunks("t", prodT, n // GRP, c0)
            drain_pending(len(pending))

            # ---- reduce r extrema -> AllGather (overlaps pass B) ----
            def reduce_gather(p):
                """acc4 cols: 0 mxA, 1 mxT(valid on HF[p]), 2 -mnA,
                3 -mnT(HF[p]); returns globx [128,4] (max over cores)."""
                hf = HF[p]
                acc4 = work.tile([128, 4], f32, name="acc4", tag=f"acc4{p}")
                nc.vector.memset(acc4[:], NEG_INF)
                mv = work.tile([128, 1], f32, name="mv", tag=f"mv{p}")
                nc.vector.tensor_reduce(acc4[:, 0:1], accmax[p][:],
                                        axis=mybir.AxisListType.X, op=maxop)
                nc.vector.tensor_reduce(mv[:], accmax["t"][:],
                                        axis=mybir.AxisListType.X, op=maxop)
                nc.vector.tensor_copy(acc4[hf, 1:2], mv[hf])
                nc.vector.tensor_reduce(mv[:], accmin[p][:],
                                        axis=mybir.AxisListType.X, op=minop)
                nc.vector.tensor_scalar(acc4[:, 2:3], mv[:], -1.0, None,
                                        mult)
                nc.vector.tensor_reduce(mv[:], accmin["t"][:],
                                        axis=mybir.AxisListType.X, op=minop)
                nc.vector.tensor_scalar(mv[:], mv[:], -1.0, None, mult)
                nc.vector.tensor_copy(acc4[hf, 3:4], mv[hf])
                accT = psum.tile([128, 512], f32, name="acc4T", tag="ps")
                nc.tensor.transpose(accT[0:4, 0:128], acc4[:, 0:4],
                                    sb_i128f[:])
                red4 = work.tile([4, 1], f32, name="red4", tag=f"red4{p}")
                nc.vector.tensor_reduce(red4[:], accT[0:4, 0:128],
                                        axis=mybir.AxisListType.X, op=maxop)
                nc.sync.dma_start(out=cc_in[p].ap(), in_=red4[:])
                if n_cores > 1:
                    nc.gpsimd.collective_compute(
                        "AllGather", mybir.AluOpType.bypass,
                        replica_groups=[list(range(n_cores))],
                        ins=[cc_in[p].ap()], outs=[cc_out[p].ap()])

            def consume_gather(p):
                globx = work.tile([128, 4], f32, name="globx",
                                  tag=f"globx{p}")
                if n_cores > 1:
                    g4n = work.tile([128, 4 * n_cores], f32, name="g4n",
                                    tag=f"g4n{p}")
                    bcast = bass.AP(tensor=cc_out[p].ap().tensor, offset=0,
                                    ap=[[0, 128], [1, 4 * n_cores]])
                    nc.sync.dma_start(out=g4n[:], in_=bcast)
                    ga = g4n[:]
                    red = bass.AP(tensor=ga.tensor, offset=ga.offset,
                                  ap=[ga.ap[0], [1, 4], [4, n_cores]])
                    nc.vector.tensor_reduce(globx[:], red,
                                            axis=mybir.AxisListType.X,
                                            op=maxop)
                else:
                    bcast = bass.AP(tensor=cc_in[p].ap().tensor, offset=0,
                                    ap=[[0, 128], [1, 4]])
                    nc.sync.dma_start(out=globx[:], in_=bcast)
                # derive svec[p]
                gmx = work.tile([128, 1], f32, name="gmx", tag=f"gmx{p}")
                gnm = work.tile([128, 1], f32, name="gnm", tag=f"gnm{p}")
                nc.vector.tensor_tensor(gmx[:], globx[:, 0:1],
                                        globx[:, 1:2], maxop)
                nc.vector.tensor_tensor(gnm[:], globx[:, 2:3],
                                        globx[:, 3:4], maxop)
                tmp = work.tile([128, 1], f32, name="tmp", tag=f"tmp{p}")
                tmp2 = work.tile([128, 1], f32, name="tmp2", tag=f"tmp2{p}")
                nc.vector.tensor_scalar(svec[(p, "mn")][:], gnm[:],
                                        -1.0, None, mult)
                nc.vector.tensor_tensor(tmp[:], gmx[:], gnm[:], addop)
                nc.vector.reciprocal(svec[(p, "inv")][:], tmp[:])
                nc.vector.tensor_tensor(svec[(p, "sc0")][:],
                                        svec[(p, "inv")][:], sb_alpha[:],
                                        mult)
                nc.vector.tensor_tensor(tmp[:], svec[(p, "mn")][:],
                                        svec[(p, "inv")][:], mult)
                nc.vector.tensor_tensor(tmp2[:], tmp[:], sb_beta[:], addop)
                nc.vector.scalar_tensor_tensor(
                    svec[(p, "bi0")][:], in0=tmp2[:], scalar=-1.0,
                    in1=sb_alpha[:], op0=mult, op1=mult)

            reduce_gather("r")

            # ================= PASS B: i-main stream =================
            for n in range(NT):
                ch, lt = n // TPC, n % TPC
                lc = slice(lt * nt_cols, (lt + 1) * nt_cols)
                cols = slice(n * nt_cols, (n + 1) * nt_cols)
                bA = psum.tile([128, nt_cols], f32, name="bA", tag="ps")
                nc.tensor.matmul(bA[:], sb_p["C_A0"][:], sA["i"][ch][:, lc],
                                 start=True, stop=False)
                nc.tensor.matmul(bA[:], sb_p["C_B0"][0:64, :],
                                 sB[ch][0:64, lc], start=False, stop=True)
                bbf_i = work.tile([128, nt_cols], bf16, name="bbf",
                                  tag="bbf_i")
                nc.scalar.copy(bbf_i[:], bA[:])
                tA = psum.tile([128, nt_cols], f32, name="tA", tag="ps")
                nc.tensor.matmul(tA[:], sb_p["MT_A0"][:], swAi_big[:, cols],
                                 start=True, stop=False)
                nc.tensor.matmul(tA[:], sb_p["MT_B0"][0:64, :],
                                 swT_big[0:64, cols], start=False,
                                 stop=True)
                nc.vector.tensor_tensor(prodA["i"][:, cols], tA[:],
                                        bbf_i[:], mult)
                drain_pending(2)
                if n % GRP == GRP - 1:
                    tree_thunks("i", prodA["i"], n // GRP,
                                (n - GRP + 1) * nt_cols)
            drain_pending(len(pending))

            consume_gather("r")       # r scalars (gather done in pass B)
            reduce_gather("i")        # i gather overlaps phase2-r
            p1i.close()               # free inputs/sw/tree/psum SBUF

            # ================= PHASE 2 =================
            gbig = p1.enter_context(tc.tile_pool(name="gbig", bufs=1))
            gp = p1.enter_context(tc.tile_pool(name="gp", bufs=3))
            ops = p1.enter_context(tc.tile_pool(name="ops", bufs=2))
            HW = 2048
            NB = b_loc // HW
            gt = {p: gbig.tile([128, b_loc], bf16, name=f"g_{p}",
                               tag=f"g_{p}")
                  for p in ("r", "i", "t")}

            def p2_block(p, src, hc):
                sn = gp.tile([128, HW], bf16, name="sn", tag="sn")
                nc.vector.tensor_scalar(sn[:], src[:, hc],
                                        svec[(p, "mn")][:],
                                        svec[(p, "inv")][:], subop, mult)
                res = gp.tile([128, HW], bf16, name="res", tag="res")
                nc.vector.tensor_tensor(res[:], sn[:], gt[p][:, hc], mult)
                if p != "t":
                    nc.vector.tensor_tensor(src[:, hc], res[:], res[:],
                                            mult)
                    return None
                # r-tail sq in place; i-tail sq written shifted up to
                # partitions 64:128 (inputs share base 0 -> legal on HW)
                nc.vector.tensor_tensor(prodT[64:128, hc], res[64:128, :],
                                        res[64:128, :], mult)
                sqi = gp.tile([128, HW], bf16, name="sqi", tag="sqi")
                nc.vector.tensor_tensor(sqi[64:128, :], res[0:64, :],
                                        res[0:64, :], mult)
                return sqi

            # phase2-r (hides the i AllGather): sigmoids then DVE blocks
            for h in range(NB):
                hc = slice(h * HW, (h + 1) * HW)
                nc.scalar.activation(gt["r"][:, hc], prodA["r"][:, hc],
                                     AF.Sigmoid, bias=svec[("r", "bi0")][:],
                                     scale=svec[("r", "sc0")][:])
            for h in range(NB):
                p2_block("r", prodA["r"], slice(h * HW, (h + 1) * HW))

            consume_gather("i")       # i scalars
            for nm in ("mn", "inv", "sc0", "bi0"):
                for p in ("r", "i"):
                    hf = HF[p]
                    nc.vector.tensor_copy(svec[("t", nm)][hf],
                                          svec[(p, nm)][hf])

            # phase2 rest: i-main + merged tail, sumsq, sqrt, stores
            for h in range(NB):
                hc = slice(h * HW, (h + 1) * HW)
                for p, src in (("i", prodA["i"]), ("t", prodT)):
                    nc.scalar.activation(gt[p][:, hc], src[:, hc],
                                         AF.Sigmoid,
                                         bias=svec[(p, "bi0")][:],
                                         scale=svec[(p, "sc0")][:])
            for h in range(NB):
                hc = slice(h * HW, (h + 1) * HW)
                p2_block("i", prodA["i"], hc)
                sqi = p2_block("t", prodT, hc)
                nc.vector.tensor_tensor(prodA["r"][:, hc], prodA["r"][:, hc],
                                        prodA["i"][:, hc], addop)
                nc.vector.tensor_tensor(prodT[64:128, hc], prodT[64:128, hc],
                                        sqi[64:128, :], addop)
                oA = ops.tile([128, HW], bf16, name="oA", tag="oA")
                nc.scalar.activation(oA[:], prodA["r"][:, hc], AF.Sqrt)
                nc.sync.dma_start(out=outA_d.ap()[:, hc], in_=oA[:])
                oT = ops.tile([128, HW], bf16, name="oT", tag="oT")
                nc.scalar.activation(oT[64:128, :], prodT[64:128, hc],
                                     AF.Sqrt)
                nc.sync.dma_start(out=outT_d.ap()[:, hc],
                                  in_=oT[64:128, :])

    nc.compile()
    return nc


def assemble_output(outA, outT):
    """[128,B]+[64,B] bf16 feature-major -> [B,181] f32 batch-major."""
    b = outA.shape[1]
    out = np.empty((b, N_GRID), np.float32)
    out[:, 0:CH0] = np.asarray(outA, np.float32).T
    out[:, CH0:N_GRID] = np.asarray(outT[CH0 - TL0:TLW], np.float32).T
    return out


def kernel(**inputs):
    s_minus = np.ascontiguousarray(inputs["s_minus"], dtype=np.float32)
    bf_vector = np.ascontiguousarray(inputs["bf_vector"], dtype=np.float32)
    alpha = np.asarray(inputs["alpha"], dtype=np.float32).reshape(-1)[0]
    beta = np.asarray(inputs["beta"], dtype=np.float32).reshape(-1)[0]

    params = build_host_params(
        np.asarray(inputs["phi1"], np.float32),
        np.asarray(inputs["phi2"], np.float32),
        np.asarray(inputs["phi3"], np.float32),
        np.asarray(inputs["phi4"], np.float32),
        np.asarray(inputs["phi5"], np.float32))

    import ml_dtypes
    s_bf_full = np.ascontiguousarray(s_minus.astype(ml_dtypes.bfloat16))
    v_bf_full = np.ascontiguousarray(bf_vector.astype(ml_dtypes.bfloat16))
    st_full = np.empty((s_minus.shape[0], 128), np.float32)
    st_full[:, 0:64] = s_minus[:, N_GRID + TL0:2 * N_GRID]   # i tail
    st_full[:, 64:128] = s_minus[:, TL0:N_GRID]              # r tail
    st_bf_full = np.ascontiguousarray(st_full.astype(ml_dtypes.bfloat16))
    b_loc = B_FULL // N_CORES
    nc = build_bass(b_loc, N_CORES)

    base = {k: np.ascontiguousarray(v) for k, v in params.items()}
    base["alpha_b"] = np.full((128, 1), alpha, np.float32)
    base["beta_b"] = np.full((128, 1), beta, np.float32)

    in_maps = []
    for c in range(N_CORES):
        m = dict(base)
        m["s_bf"] = s_bf_full[c * b_loc:(c + 1) * b_loc]
        m["v_bf"] = v_bf_full[c * b_loc:(c + 1) * b_loc]
        m["st_bf"] = st_bf_full[c * b_loc:(c + 1) * b_loc]
        in_maps.append(m)

    from concourse.bass_utils import run_bass_kernel_spmd
    res = run_bass_kernel_spmd(nc, in_maps, core_ids=list(range(N_CORES)))
    global LAST_EXEC_NS
    LAST_EXEC_NS = res.exec_time_ns
    if LAST_EXEC_NS is not None:
        print(f"HW exec time: {LAST_EXEC_NS} ns")
    return np.concatenate(
        [assemble_output(r["outA"], r["outT"]) for r in res.results], axis=0)
